# revision 27
# baseline (speedup 1.0000x reference)
"""Trainium2 Bass kernel for nn_ContrastiveLoss (B=2048, D=4096, C=1000, 8 cores).

loss = CE(y_preds, y_true) + pos + neg, with
  pos = mean over same-label pairs i<j of (1 - cos(x_i, x_j))
  neg = mean over the 16 pairs (0,j), j=1..16 of relu(cos(x_0, x_j))

Math refactor (exact up to fp rounding): with xn_i = x_i / max(|x_i|, eps),
  sum_{i<j, y_i=y_j} cos_ij = (||G||_F^2 - sum_i |xn_i|^2) / 2,
  where G[c] = sum_{i: y_i=c} xn_i  (per-class sums).
No BxB similarity matrix needed. Classes are LPT-balanced onto cores
(<=256 rows, <=128 classes per core -> 2 row tiles, no padding). The
row normalization folds into the one-hot: G = A'^T X with
A'[i,c] = (y_i==c) * 64/|x_i| (x in fp8e4m3, so inv is x64-scaled into
fp8's sweet spot; ||G||^2 comes out 4096x and the host divides).

Per-engine split: PE does the one-hot matmuls, the 17x17 neg Gram and
the ||G||^2 column-fold; ACT squares (n2 tail, G^2) plus CE exp/ln and
the exp(-0.5 ln n2) inverse norms (activation tables patched to the
one set that holds exp+ln+square); DVE does the n2 bulk via stt and
the small glue; GpSimd does the CE label-gather (indirect_copy) and
the neg-block squares. All DMAs ride the two HWDGE queues, issued
before any compute. Host-side work is layout only (fp16/fp8 casts,
bucketing, transposing the 17-row block); partials combine on the host
(~20 scalar flops).
"""

import numpy as np
import ml_dtypes

import concourse.bacc as bacc
import concourse.tile as tile
from concourse.tile_rust import add_dep_helper
from concourse import mybir
from concourse import bass_utils
from concourse import hw_specs as _hw_specs

# Restrict bacc's activation-table chooser to the one set that contains
# every ACT function this kernel uses (exp, ln, square) so a single
# ACT_TABLE_LOAD suffices (walrus's own chooser splits exp and ln into
# different sets and thrashes ~1.3us per switch).
_ORIG_GAT = _hw_specs.get_activation_tables
_KEEP_SETS = ("natural_log_exp_and_others", "abs_reciprocal_sqrt_and_small")


def _gat_keep_sets(arch):
    t = _ORIG_GAT(arch)
    if not all(k in t for k in _KEEP_SETS):
        return t
    return {k: (v if k in _KEEP_SETS else set()) for k, v in t.items()}


bacc.get_activation_tables = _gat_keep_sets

F32 = mybir.dt.float32
F16 = mybir.dt.float16
F8 = mybir.dt.float8e4
U16 = mybir.dt.uint16
ALU = mybir.AluOpType
ACTF = mybir.ActivationFunctionType
AX = mybir.AxisListType

B, D, C = 2048, 4096, 1000
NCORES = 8
NCLS = 128                     # one-hot width (classes per core cap)
CE_ROWS = B // NCORES          # 256
CE_T = CE_ROWS // 128          # 2
KNEG = 17                      # rows 0..16 for the negative pairs
KD = D // 128                  # 32 contraction chunks for the neg Gram
NEGW = KD * KNEG               # 544
SPL = 2816                     # n2 split: DVE stt does [0:SPL], ACT the rest
NPH = 4                        # G psum phases over D
PHW = D // NPH                 # 2048
LN64 = float(np.log(64.0))
GSCALE = 4096.0                # ||G||^2 scale from the x64 one-hot

# out vector layout: [ce0, ce1, g2, 0, 0, 0, m2, negsum]
OUTW = 8


def build_nc(nt=2):
    nc = bacc.Bacc("TRN2", target_bir_lowering=False)

    xb_d = nc.dram_tensor("xb", [nt, 128, D], F8, kind="ExternalInput")
    yp_d = nc.dram_tensor("yp", [CE_T, 128, C], F16, kind="ExternalInput")
    sm_d = nc.dram_tensor("sm", [128, nt + 2], F32, kind="ExternalInput")
    xng_d = nc.dram_tensor("xng", [128, NEGW], F16, kind="ExternalInput")
    out_d = nc.dram_tensor("out", [1, OUTW], F32, kind="ExternalOutput")

    with tile.TileContext(nc) as tc:
        with (
            tc.tile_pool(name="singles", bufs=1) as singles,
            tc.tile_pool(name="xpool", bufs=nt) as xpool,
            tc.tile_pool(name="apool", bufs=nt) as apool,
            tc.tile_pool(name="ppool", bufs=nt) as ppool,
            tc.tile_pool(name="junka", bufs=2) as junka,
            tc.tile_pool(name="junkb", bufs=2) as junkb,
            tc.tile_pool(name="cepool", bufs=2) as cepool,
            tc.tile_pool(name="cejunk", bufs=2) as cejunk,
            tc.tile_pool(name="gsqp", bufs=2) as gsqp,
            tc.tile_pool(name="small", bufs=8) as small,
            tc.tile_pool(name="psG", bufs=2, space="PSUM") as psG,
            tc.tile_pool(name="psS", bufs=3, space="PSUM") as psS,
            tc.tile_pool(name="psW", bufs=1, space="PSUM") as psW,
        ):
            # ---- input DMAs first. x tiles lead on the two HWDGE
            # queues; the rest rides behind / on the SWDGE queue.
            x_tiles = []
            for t in range(nt):
                xt = xpool.tile([128, D], F8, tag="xt")
                (nc.sync, nc.scalar)[t % 2].dma_start(out=xt[:], in_=xb_d[t])
                x_tiles.append(xt)
            smf = singles.tile([128, nt + 2], F32)
            nc.sync.dma_start(out=smf[:], in_=sm_d[:])
            xng = singles.tile([128, NEGW], F16)
            nc.scalar.dma_start(out=xng[:], in_=xng_d[:])
            z_tiles = []
            for i in range(CE_T):
                zt = cepool.tile([128, C], F16, tag="zt")
                (nc.sync, nc.scalar)[i % 2].dma_start(out=zt[:], in_=yp_d[i])
                z_tiles.append(zt)

            # ---- constants ----
            iota_cls = singles.tile([128, NCLS], F16)
            nc.gpsimd.iota(iota_cls[:], pattern=[[1, NCLS]], base=0,
                           channel_multiplier=0,
                           allow_small_or_imprecise_dtypes=True)
            iota_ce = singles.tile([128, C], F16)
            nc.gpsimd.iota(iota_ce[:], pattern=[[1, C]], base=0,
                           channel_multiplier=0,
                           allow_small_or_imprecise_dtypes=True)
            ones_8 = singles.tile([128, 1], F8)
            nc.vector.memset(ones_8[:], 1.0)
            ones_f = singles.tile([128, 1], F32)
            nc.vector.memset(ones_f[:], 1.0)
            ones_h = singles.tile([128, 1], F16)
            nc.vector.memset(ones_h[:], 1.0)
            V = singles.tile([128, OUTW], F32)
            nc.vector.memset(V[:], 0.0)
            out_sb = singles.tile([1, OUTW], F32)
            nc.vector.memset(out_sb[:], 0.0)


            # seed ACT's first table load with the abs-rsqrt set (the one
            # n2/inv needs first); CE's exp/ln set loads later, off the
            # critical path
            dummy = small.tile([1, 1], F32, tag="dummy")
            nc.scalar.activation(out=dummy[:], in_=ones_f[0:1, 0:1],
                                 func=ACTF.Abs_reciprocal_sqrt)

            # ---- negative pairs: 17x17 Gram in K-layout (early PE work) ----
            g17 = psS.tile([KNEG, KNEG], F32, tag="ps_small")
            for k in range(KD):
                sl = xng[:, k * KNEG : (k + 1) * KNEG]
                nc.tensor.matmul(g17[:], sl, sl, start=(k == 0),
                                 stop=(k == KD - 1))
            # gpsimd squares for the neg norms (idle engine, runs early)
            sqng = singles.tile([128, NEGW], F16)
            nc.gpsimd.tensor_mul(sqng[:], xng[:], xng[:])

            # ---- per-tile: row norms, x64-scaled one-hot ----
            ap_tiles = []
            mcnt = psS.tile([128, 1], F32, tag="ps_small")
            warm = psW.tile([1, 1], F32)
            wfill = [0]

            def warm_mm(rhs_ap):
                nc.tensor.matmul(warm[:], ones_f[:], rhs_ap,
                                 start=(wfill[0] == 0), stop=(wfill[0] == 3))
                wfill[0] += 1

            prev_exp = None
            prev_ap = None
            for t in range(nt):
                xt = x_tiles[t]
                n2a = small.tile([128, 1], F32, tag="n2a")
                ja = junka.tile([128, SPL], F16, tag="ja")
                i_stt = nc.vector.scalar_tensor_tensor(
                    out=ja[:], in0=xt[:, 0:SPL], scalar=0.0,
                    in1=xt[:, 0:SPL], op0=ALU.add, op1=ALU.mult,
                    accum_out=n2a[:])
                if prev_ap is not None:
                    # DVE runs the previous tile's one-hot build before
                    # grinding this tile's norm (ordering-only dep)
                    add_dep_helper(i_stt.ins, prev_ap.ins, sync=False,
                                   reason="A' priority over next n2")
                warm_mm(n2a[:])
                n2b = small.tile([128, 1], F32, tag="n2b")
                jb = junkb.tile([128, D - SPL], F16, tag="jb")
                i_sq = nc.scalar.activation(out=jb[:], in_=xt[:, SPL:D],
                                            func=ACTF.Square, scale=1.0 / 64.0,
                                            accum_out=n2b[:])
                if prev_exp is not None:
                    # keep ACT on the inv chain: t1's bulk square yields
                    # to t0's tiny rsqrt (ordering-only dep)
                    add_dep_helper(i_sq.ins, prev_exp.ins, sync=False,
                                   reason="inv chain priority")
                # inv64 = rsqrt(n2a/4096 + n2b) = 64 / |x_row|
                invc = small.tile([128, 1], F32, tag="invc")
                i_exp = nc.scalar.activation(out=invc[:], in_=n2a[:],
                                             func=ACTF.Abs_reciprocal_sqrt,
                                             scale=1.0 / 4096.0,
                                             bias=n2b[:])
                prev_exp = i_exp
                at = apool.tile([128, NCLS], F8, tag="a")
                nc.vector.tensor_scalar(out=at[:], in0=iota_cls[:],
                                        scalar1=smf[:, t : t + 1],
                                        scalar2=None, op0=ALU.is_equal)
                apt = ppool.tile([128, NCLS], F8, tag="ap")
                i_ap = nc.vector.tensor_scalar(out=apt[:], in0=iota_cls[:],
                                               scalar1=smf[:, t : t + 1],
                                               scalar2=invc[:],
                                               op0=ALU.is_equal,
                                               op1=ALU.mult)
                prev_ap = i_ap
                ap_tiles.append(apt)
                nc.tensor.matmul(mcnt[:], at[:], ones_8[:], start=(t == 0),
                                 stop=(t == nt - 1))
            mcs = small.tile([128, 1], F32, tag="mcs")
            nc.vector.tensor_copy(out=mcs[:], in_=mcnt[:])
            nc.vector.tensor_mul(V[:, 6:7], mcs[:], mcs[:])

            # ---- cross entropy shard ----
            se2 = small.tile([128, CE_T], F32, tag="se2")
            ls2 = small.tile([128, CE_T], F32, tag="ls2")
            zys = []
            for i in range(CE_T):
                zt = z_tiles[i]
                ez = cejunk.tile([128, C], F16, tag="ez")
                i_ce = nc.scalar.activation(out=ez[:], in_=zt[:],
                                            func=ACTF.Exp,
                                            accum_out=se2[:, i : i + 1])
                if i == 0 and prev_exp is not None:
                    add_dep_helper(i_ce.ins, prev_exp.ins, sync=False,
                                   reason="inv chain priority over CE")
                pz = cejunk.tile([128, C], F16, tag="pz")
                zy = small.tile([128, 1], F32, tag="zy")
                i_zy = nc.vector.scalar_tensor_tensor(
                    out=pz[:], in0=iota_ce[:],
                    scalar=smf[:, nt + i : nt + i + 1], in1=zt[:],
                    op0=ALU.is_equal, op1=ALU.mult, accum_out=zy[:])
                if i == 0 and prev_ap is not None:
                    add_dep_helper(i_zy.ins, prev_ap.ins, sync=False,
                                   reason="A' priority over CE gather")
                warm_mm(zy[:])
                zys.append(zy)
            nc.scalar.activation(out=ls2[:], in_=se2[:], func=ACTF.Ln)
            for i in range(CE_T):
                nc.vector.tensor_sub(V[:, i : i + 1], ls2[:, i : i + 1],
                                     zys[i][:])

            # ---- G accumulation in 2 D-phases; ||G||^2 via square + fold ----
            v512 = psS.tile([1, 512], F32, tag="ps_small")
            for p in range(NPH):
                gh = psG.tile([128, PHW], F32, tag="gh")
                for t in range(nt):
                    for s in range(PHW // 512):
                        lo = p * PHW + s * 512
                        nc.tensor.matmul(
                            gh[:, s * 512 : (s + 1) * 512],
                            ap_tiles[t][:], x_tiles[t][:, lo : lo + 512],
                            start=(t == 0), stop=(t == nt - 1))
                gs = gsqp.tile([128, PHW], F16, tag="gs")
                nc.scalar.activation(out=gs[:], in_=gh[:], func=ACTF.Square)
                for s in range(PHW // 512):
                    nc.tensor.matmul(
                        v512[:], ones_h[:], gs[:, s * 512 : (s + 1) * 512],
                        start=(p == 0 and s == 0),
                        stop=(p == NPH - 1 and s == PHW // 512 - 1))
            nc.vector.reduce_sum(out=out_sb[0:1, 2:3], in_=v512[0:1, :],
                                 axis=AX.X)

            # ---- neg finale (non-critical; keeps DVE/ACT free early) ----
            red17 = singles.tile([128, KNEG], F32)
            nc.vector.reduce_sum(out=red17[:],
                                 in_=sqng[:].rearrange("p (k j) -> p j k",
                                                       k=KD, j=KNEG),
                                 axis=AX.X)
            nsq = psS.tile([1, KNEG], F32, tag="ps_small")
            nc.tensor.matmul(nsq[:], ones_f[:], red17[:], start=True,
                             stop=True)
            # negsum = sum_j relu(g17[0,j]) / (n_0 n_j), j=1..16
            lnn = small.tile([1, KNEG], F32, tag="lnn")
            nc.scalar.activation(out=lnn[:], in_=nsq[:], func=ACTF.Ln)
            lns = small.tile([1, KNEG], F32, tag="lns")
            nc.vector.tensor_scalar_add(lns[:], lnn[:], lnn[0:1, 0:1])
            inv17 = small.tile([1, KNEG], F32, tag="inv17")
            nc.scalar.activation(out=inv17[:], in_=lns[:], func=ACTF.Exp,
                                 scale=-0.5)
            negs = small.tile([1, KNEG - 1], F32, tag="negs")
            nc.vector.scalar_tensor_tensor(
                out=negs[:], in0=g17[0:1, 1:KNEG], scalar=0.0,
                in1=inv17[0:1, 1:KNEG], op0=ALU.max, op1=ALU.mult,
                accum_out=out_sb[0:1, 7:8])

            # ---- partition-reduce V via ones matmul, assemble output ----
            red = psS.tile([1, OUTW], F32, tag="ps_small")
            nc.tensor.matmul(red[:], ones_f[:], V[:], start=True, stop=True)
            nc.vector.tensor_copy(out=out_sb[:, 3:4], in_=warm[0:1, 0:1])
            nc.vector.tensor_copy(out=out_sb[:, 0:2], in_=red[0:1, 0:2])
            nc.vector.tensor_copy(out=out_sb[:, 6:7], in_=red[0:1, 6:7])
            nc.sync.dma_start(out=out_d[:], in_=out_sb[:])

    nc.finalize()
    return nc


_NC_CACHE = {}


def _get_nc(nt):
    if nt not in _NC_CACHE:
        _NC_CACHE[nt] = build_nc(nt)
    return _NC_CACHE[nt]


def _balance_classes(y):
    """LPT-assign classes to cores; returns (assign[C], loads[NCORES])."""
    import heapq
    cnt = np.bincount(y, minlength=C)
    assign = np.full(C, -1, dtype=np.int64)
    heap = [(0, 0, k) for k in range(NCORES)]  # (load, nclasses, core)
    heapq.heapify(heap)
    skipped = []
    for c in np.argsort(-cnt, kind="stable"):
        if cnt[c] == 0:
            break
        load, ncl, k = heapq.heappop(heap)
        if ncl >= NCLS:  # bin full of classes; try others
            skipped.append((load, ncl, k))
            while heap and heap[0][1] >= NCLS:
                skipped.append(heapq.heappop(heap))
            if not heap:
                raise RuntimeError("class balancing failed")
            load, ncl, k = heapq.heappop(heap)
        assign[c] = k
        heapq.heappush(heap, (load + int(cnt[c]), ncl + 1, k))
        for s in skipped:
            heapq.heappush(heap, s)
        skipped = []
    loads = np.zeros(NCORES, dtype=np.int64)
    np.add.at(loads, assign[y], 1)
    return assign, loads


def make_in_maps(xs, y_preds, y_true, nt):
    rb = nt * 128
    xs16 = np.asarray(xs, dtype=np.float16)
    xs8 = np.asarray(xs, dtype=np.float32).astype(ml_dtypes.float8_e4m3)
    yp16 = np.asarray(y_preds, dtype=np.float16)
    y = np.asarray(y_true).astype(np.int64).ravel()
    assert xs8.shape == (B, D) and yp16.shape == (B, C) and y.shape == (B,)

    assign, loads = _balance_classes(y)
    assert loads.max() <= rb, f"bucket overflow: {loads.max()} > {rb}"
    lidx = np.zeros(C, dtype=np.int64)
    for k in range(NCORES):
        cls_k = np.nonzero(assign == k)[0]
        lidx[cls_k] = np.arange(len(cls_k))

    # neg block (fp16): xng[p, k*17+j] = xs[j, k*128+p]
    xng = np.ascontiguousarray(
        xs16[:KNEG].T.reshape(KD, 128, KNEG).transpose(1, 0, 2)
    ).reshape(128, NEGW)

    row_core = assign[y]
    in_maps = []
    for k in range(NCORES):
        rows = np.nonzero(row_core == k)[0]
        nk = len(rows)
        # pad rows are ONES so ln(n2) stays finite; yb=-1 zeroes them out
        xb = np.ones((rb, D), dtype=ml_dtypes.float8_e4m3)
        xb[:nk] = xs8[rows]
        yb = np.full(rb, -1.0, dtype=np.float32)
        yb[:nk] = lidx[y[rows]].astype(np.float32)
        yt = y[k * CE_ROWS : (k + 1) * CE_ROWS].astype(np.float32)
        sm = np.empty((128, nt + 2), dtype=np.float32)
        for t in range(nt):
            sm[:, t] = yb[t * 128 : (t + 1) * 128]
        sm[:, nt] = yt[0:128]
        sm[:, nt + 1] = yt[128:256]
        in_maps.append({
            "xb": xb.reshape(nt, 128, D),
            "yp": yp16[k * CE_ROWS : (k + 1) * CE_ROWS].reshape(CE_T, 128, C),
            "sm": sm,
            "xng": xng,
        })
    return in_maps


def combine(outs):
    """outs: [NCORES][1, OUTW] partial vectors -> final loss scalar."""
    o = np.stack([np.asarray(x, dtype=np.float64).ravel() for x in outs])
    ce_sum = o[:, 0].sum() + o[:, 1].sum()
    g2 = o[:, 2].sum() / GSCALE
    m2 = o[:, 6].sum()
    neg = o[0, 7]
    loss_ce = ce_sum / B
    cnt = (m2 - B) / 2.0
    sum_s = (g2 - B) / 2.0
    pos_sum = cnt - sum_s
    loss_pos = pos_sum / max(cnt, 1.0) if cnt > 0 else 0.0
    loss_neg = neg / (KNEG - 1)
    return np.array(loss_ce + loss_pos + loss_neg, dtype=np.float32)


def kernel(xs, y_preds, y_true, _trace=False):
    y = np.asarray(y_true).astype(np.int64).ravel()
    _, loads = _balance_classes(y)
    nt = max(2, -(-int(loads.max()) // 128))
    nc = _get_nc(nt)
    in_maps = make_in_maps(xs, y_preds, y_true, nt)
    kw = {}
    if _trace:
        import os
        td = "/tmp/trace_out"
        os.makedirs(td, exist_ok=True)
        kw["tmpdir"] = td
    res = bass_utils.run_bass_kernel_spmd(
        nc, in_maps, core_ids=list(range(NCORES)), trace=_trace, **kw,
    )
    loss = combine([r["out"] for r in res.results])
    if _trace:
        return loss, res
    return loss


# revision 28
# speedup vs baseline: 1.1231x; 1.1231x over previous
"""Trainium2 Bass kernel for nn_ContrastiveLoss (B=2048, D=4096, C=1000, 8 cores).

loss = CE(y_preds, y_true) + pos + neg, with
  pos = mean over same-label pairs i<j of (1 - cos(x_i, x_j))
  neg = mean over the 16 pairs (0,j), j=1..16 of relu(cos(x_0, x_j))

Math refactor (exact up to fp rounding): with xn_i = x_i / max(|x_i|, eps),
  sum_{i<j, y_i=y_j} cos_ij = (||G||_F^2 - sum_i |xn_i|^2) / 2,
  where G[c] = sum_{i: y_i=c} xn_i  (per-class sums).
No BxB similarity matrix needed. Classes are LPT-balanced onto cores
(<=256 rows, <=128 classes per core -> 2 row tiles, no padding). The
row normalization folds into the one-hot: G = A'^T X with
A'[i,c] = (y_i==c) * 64/|x_i| (x in fp8e4m3, so inv is x64-scaled into
fp8's sweet spot; ||G||^2 comes out 4096x and the host divides).

Per-engine split: PE does the one-hot matmuls, the 17x17 neg Gram and
the ||G||^2 column-fold; ACT squares (n2 tail, G^2) plus CE exp/ln and
the exp(-0.5 ln n2) inverse norms (activation tables patched to the
one set that holds exp+ln+square); DVE does the n2 bulk via stt and
the small glue; GpSimd does the CE label-gather (indirect_copy) and
the neg-block squares. All DMAs ride the two HWDGE queues, issued
before any compute. Host-side work is layout only (fp16/fp8 casts,
bucketing, transposing the 17-row block); partials combine on the host
(~20 scalar flops).
"""

import numpy as np
import ml_dtypes

import concourse.bacc as bacc
import concourse.tile as tile
from concourse.tile_rust import add_dep_helper
from concourse import mybir
from concourse import bass_utils
from concourse import hw_specs as _hw_specs

# Restrict bacc's activation-table chooser to the one set that contains
# every ACT function this kernel uses (exp, ln, square) so a single
# ACT_TABLE_LOAD suffices (walrus's own chooser splits exp and ln into
# different sets and thrashes ~1.3us per switch).
_ORIG_GAT = _hw_specs.get_activation_tables
_KEEP_SETS = ("natural_log_exp_and_others", "abs_reciprocal_sqrt_and_small")


def _gat_keep_sets(arch):
    t = _ORIG_GAT(arch)
    if not all(k in t for k in _KEEP_SETS):
        return t
    return {k: (v if k in _KEEP_SETS else set()) for k, v in t.items()}


bacc.get_activation_tables = _gat_keep_sets

F32 = mybir.dt.float32
F16 = mybir.dt.float16
F8 = mybir.dt.float8e4
U16 = mybir.dt.uint16
ALU = mybir.AluOpType
ACTF = mybir.ActivationFunctionType
AX = mybir.AxisListType

B, D, C = 2048, 4096, 1000
NCORES = 8
NCLS = 128                     # one-hot width (classes per core cap)
CE_ROWS = B // NCORES          # 256
CE_T = CE_ROWS // 128          # 2
KNEG = 17                      # rows 0..16 for the negative pairs
KD = D // 128                  # 32 contraction chunks for the neg Gram
NEGW = KD * KNEG               # 544
SPL = 2048                     # n2 split: DVE stt does [0:SPL], ACT the rest
NPH = 4                        # G psum phases over D
PHW = D // NPH                 # 2048
LN64 = float(np.log(64.0))
GSCALE = 4096.0                # ||G||^2 scale from the x64 one-hot

# out vector layout: [ce0, ce1, g2, 0, 0, 0, m2, negsum]
OUTW = 8


def build_nc(nt=2):
    nc = bacc.Bacc("TRN2", target_bir_lowering=False)

    xb_d = nc.dram_tensor("xb", [nt, 128, D], F8, kind="ExternalInput")
    yp_d = nc.dram_tensor("yp", [CE_T, 128, C], F16, kind="ExternalInput")
    sm_d = nc.dram_tensor("sm", [128, nt + 2], F32, kind="ExternalInput")
    xng_d = nc.dram_tensor("xng", [128, NEGW], F16, kind="ExternalInput")
    out_d = nc.dram_tensor("out", [1, OUTW], F32, kind="ExternalOutput")

    with tile.TileContext(nc) as tc:
        with (
            tc.tile_pool(name="singles", bufs=1) as singles,
            tc.tile_pool(name="xpool", bufs=nt) as xpool,
            tc.tile_pool(name="apool", bufs=nt) as apool,
            tc.tile_pool(name="ppool", bufs=nt) as ppool,
            tc.tile_pool(name="junka", bufs=2) as junka,
            tc.tile_pool(name="junkb", bufs=2) as junkb,
            tc.tile_pool(name="cepool", bufs=2) as cepool,
            tc.tile_pool(name="cejunk", bufs=2) as cejunk,
            tc.tile_pool(name="gsqp", bufs=2) as gsqp,
            tc.tile_pool(name="small", bufs=8) as small,
            tc.tile_pool(name="psG", bufs=2, space="PSUM") as psG,
            tc.tile_pool(name="psS", bufs=3, space="PSUM") as psS,
            tc.tile_pool(name="psW", bufs=1, space="PSUM") as psW,
        ):
            # ---- input DMAs first. x tiles lead on the two HWDGE
            # queues; the rest rides behind / on the SWDGE queue.
            x_tiles = []
            for t in range(nt):
                xt = xpool.tile([128, D], F8, tag="xt")
                (nc.sync, nc.scalar)[t % 2].dma_start(out=xt[:], in_=xb_d[t])
                x_tiles.append(xt)
            smf = singles.tile([128, nt + 2], F32)
            nc.sync.dma_start(out=smf[:], in_=sm_d[:])
            xng = singles.tile([128, NEGW], F16)
            nc.scalar.dma_start(out=xng[:], in_=xng_d[:])
            z_tiles = []
            for i in range(CE_T):
                zt = cepool.tile([128, C], F16, tag="zt")
                (nc.sync, nc.scalar)[i % 2].dma_start(out=zt[:], in_=yp_d[i])
                z_tiles.append(zt)

            # ---- constants ----
            iota_cls = singles.tile([128, NCLS], F16)
            nc.gpsimd.iota(iota_cls[:], pattern=[[1, NCLS]], base=0,
                           channel_multiplier=0,
                           allow_small_or_imprecise_dtypes=True)
            iota_ce = singles.tile([128, C], F16)
            nc.gpsimd.iota(iota_ce[:], pattern=[[1, C]], base=0,
                           channel_multiplier=0,
                           allow_small_or_imprecise_dtypes=True)
            ones_8 = singles.tile([128, 1], F8)
            nc.vector.memset(ones_8[:], 1.0)
            ones_f = singles.tile([128, 1], F32)
            nc.vector.memset(ones_f[:], 1.0)
            ones_h = singles.tile([128, 1], F16)
            nc.vector.memset(ones_h[:], 1.0)
            V = singles.tile([128, OUTW], F32)
            nc.vector.memset(V[:], 0.0)
            out_sb = singles.tile([1, OUTW], F32)
            nc.vector.memset(out_sb[:], 0.0)


            # seed ACT's first table load with the abs-rsqrt set (the one
            # n2/inv needs first); CE's exp/ln set loads later, off the
            # critical path
            dummy = small.tile([1, 1], F32, tag="dummy")
            nc.scalar.activation(out=dummy[:], in_=ones_f[0:1, 0:1],
                                 func=ACTF.Abs_reciprocal_sqrt)

            # ---- negative pairs: 17x17 Gram in K-layout (early PE work) ----
            g17 = psS.tile([KNEG, KNEG], F32, tag="ps_small")
            for k in range(KD):
                sl = xng[:, k * KNEG : (k + 1) * KNEG]
                nc.tensor.matmul(g17[:], sl, sl, start=(k == 0),
                                 stop=(k == KD - 1))
            # gpsimd squares for the neg norms (idle engine, runs early)
            sqng = singles.tile([128, NEGW], F16)
            nc.gpsimd.tensor_mul(sqng[:], xng[:], xng[:])

            # ---- per-tile: row norms, x64-scaled one-hot ----
            ap_tiles = []
            mcnt = psS.tile([128, 1], F32, tag="ps_small")
            warm = psW.tile([1, 1], F32)
            wfill = [0]

            def warm_mm(rhs_ap):
                nc.tensor.matmul(warm[:], ones_f[:], rhs_ap,
                                 start=(wfill[0] == 0), stop=(wfill[0] == 3))
                wfill[0] += 1

            prev_exp = None
            prev_ap = None
            for t in range(nt):
                xt = x_tiles[t]
                n2a = small.tile([128, 1], F32, tag="n2a")
                ja = junka.tile([128, SPL], F16, tag="ja")
                i_stt = nc.vector.scalar_tensor_tensor(
                    out=ja[:], in0=xt[:, 0:SPL], scalar=0.0,
                    in1=xt[:, 0:SPL], op0=ALU.add, op1=ALU.mult,
                    accum_out=n2a[:])
                if prev_ap is not None:
                    # DVE runs the previous tile's one-hot build before
                    # grinding this tile's norm (ordering-only dep)
                    add_dep_helper(i_stt.ins, prev_ap.ins, sync=False,
                                   reason="A' priority over next n2")
                warm_mm(n2a[:])
                n2b = small.tile([128, 1], F32, tag="n2b")
                jb = junkb.tile([128, D - SPL], F16, tag="jb")
                i_sq = nc.scalar.activation(out=jb[:], in_=xt[:, SPL:D],
                                            func=ACTF.Square, scale=1.0 / 64.0,
                                            accum_out=n2b[:])
                if prev_exp is not None:
                    # keep ACT on the inv chain: t1's bulk square yields
                    # to t0's tiny rsqrt (ordering-only dep)
                    add_dep_helper(i_sq.ins, prev_exp.ins, sync=False,
                                   reason="inv chain priority")
                # inv64 = rsqrt(n2a/4096 + n2b) = 64 / |x_row|
                invc = small.tile([128, 1], F32, tag="invc")
                i_exp = nc.scalar.activation(out=invc[:], in_=n2a[:],
                                             func=ACTF.Abs_reciprocal_sqrt,
                                             scale=1.0 / 4096.0,
                                             bias=n2b[:])
                prev_exp = i_exp
                at = apool.tile([128, NCLS], F8, tag="a")
                nc.vector.tensor_scalar(out=at[:], in0=iota_cls[:],
                                        scalar1=smf[:, t : t + 1],
                                        scalar2=None, op0=ALU.is_equal)
                apt = ppool.tile([128, NCLS], F8, tag="ap")
                i_ap = nc.vector.tensor_scalar(out=apt[:], in0=iota_cls[:],
                                               scalar1=smf[:, t : t + 1],
                                               scalar2=invc[:],
                                               op0=ALU.is_equal,
                                               op1=ALU.mult)
                prev_ap = i_ap
                ap_tiles.append(apt)
                nc.tensor.matmul(mcnt[:], at[:], ones_8[:], start=(t == 0),
                                 stop=(t == nt - 1))
            mcs = small.tile([128, 1], F32, tag="mcs")
            nc.vector.tensor_copy(out=mcs[:], in_=mcnt[:])
            nc.vector.tensor_mul(V[:, 6:7], mcs[:], mcs[:])

            # ---- cross entropy shard ----
            se2 = small.tile([128, CE_T], F32, tag="se2")
            ls2 = small.tile([128, CE_T], F32, tag="ls2")
            zys = []
            for i in range(CE_T):
                zt = z_tiles[i]
                ez = cejunk.tile([128, C], F16, tag="ez")
                i_ce = nc.scalar.activation(out=ez[:], in_=zt[:],
                                            func=ACTF.Exp,
                                            accum_out=se2[:, i : i + 1])
                if i == 0 and prev_exp is not None:
                    add_dep_helper(i_ce.ins, prev_exp.ins, sync=False,
                                   reason="inv chain priority over CE")
                pz = cejunk.tile([128, C], F16, tag="pz")
                zy = small.tile([128, 1], F32, tag="zy")
                i_zy = nc.vector.scalar_tensor_tensor(
                    out=pz[:], in0=iota_ce[:],
                    scalar=smf[:, nt + i : nt + i + 1], in1=zt[:],
                    op0=ALU.is_equal, op1=ALU.mult, accum_out=zy[:])
                if i == 0 and prev_ap is not None:
                    add_dep_helper(i_zy.ins, prev_ap.ins, sync=False,
                                   reason="A' priority over CE gather")
                last_zy = i_zy
                warm_mm(zy[:])
                zys.append(zy)
            nc.scalar.activation(out=ls2[:], in_=se2[:], func=ACTF.Ln)
            for i in range(CE_T):
                nc.vector.tensor_sub(V[:, i : i + 1], ls2[:, i : i + 1],
                                     zys[i][:])

            # ---- G accumulation in 2 D-phases; ||G||^2 via square + fold ----
            v512 = psS.tile([1, 512], F32, tag="ps_small")
            for p in range(NPH):
                gh = psG.tile([128, PHW], F32, tag="gh")
                for t in range(nt):
                    for s in range(PHW // 512):
                        lo = p * PHW + s * 512
                        nc.tensor.matmul(
                            gh[:, s * 512 : (s + 1) * 512],
                            ap_tiles[t][:], x_tiles[t][:, lo : lo + 512],
                            start=(t == 0), stop=(t == nt - 1))
                gs = gsqp.tile([128, PHW], F16, tag="gs")
                if p == 0:
                    gc = gsqp.tile([128, PHW], F16, tag="gc")
                    nc.vector.tensor_copy(out=gc[:], in_=gh[:])
                    nc.vector.tensor_mul(gs[:], gc[:], gc[:])
                else:
                    nc.scalar.activation(out=gs[:], in_=gh[:],
                                         func=ACTF.Square)
                for s in range(PHW // 512):
                    nc.tensor.matmul(
                        v512[:], ones_h[:], gs[:, s * 512 : (s + 1) * 512],
                        start=(p == 0 and s == 0),
                        stop=(p == NPH - 1 and s == PHW // 512 - 1))
            nc.vector.reduce_sum(out=out_sb[0:1, 2:3], in_=v512[0:1, :],
                                 axis=AX.X)

            # ---- neg finale (non-critical; keeps DVE/ACT free early) ----
            red17 = singles.tile([128, KNEG], F32)
            i_r17 = nc.vector.reduce_sum(out=red17[:],
                                 in_=sqng[:].rearrange("p (k j) -> p j k",
                                                       k=KD, j=KNEG),
                                 axis=AX.X)
            add_dep_helper(i_r17.ins, last_zy.ins, sync=False,
                           reason="neg finale after CE gathers")
            nsq = psS.tile([1, KNEG], F32, tag="ps_small")
            nc.tensor.matmul(nsq[:], ones_f[:], red17[:], start=True,
                             stop=True)
            # negsum = sum_j relu(g17[0,j]) / (n_0 n_j), j=1..16
            lnn = small.tile([1, KNEG], F32, tag="lnn")
            nc.scalar.activation(out=lnn[:], in_=nsq[:], func=ACTF.Ln)
            lns = small.tile([1, KNEG], F32, tag="lns")
            nc.vector.tensor_scalar_add(lns[:], lnn[:], lnn[0:1, 0:1])
            inv17 = small.tile([1, KNEG], F32, tag="inv17")
            nc.scalar.activation(out=inv17[:], in_=lns[:], func=ACTF.Exp,
                                 scale=-0.5)
            negs = small.tile([1, KNEG - 1], F32, tag="negs")
            nc.vector.scalar_tensor_tensor(
                out=negs[:], in0=g17[0:1, 1:KNEG], scalar=0.0,
                in1=inv17[0:1, 1:KNEG], op0=ALU.max, op1=ALU.mult,
                accum_out=out_sb[0:1, 7:8])

            # ---- partition-reduce V via ones matmul, assemble output ----
            red = psS.tile([1, OUTW], F32, tag="ps_small")
            nc.tensor.matmul(red[:], ones_f[:], V[:], start=True, stop=True)
            nc.vector.tensor_copy(out=out_sb[:, 3:4], in_=warm[0:1, 0:1])
            nc.vector.tensor_copy(out=out_sb[:, 0:2], in_=red[0:1, 0:2])
            nc.vector.tensor_copy(out=out_sb[:, 6:7], in_=red[0:1, 6:7])
            nc.sync.dma_start(out=out_d[:], in_=out_sb[:])

    nc.finalize()
    return nc


_NC_CACHE = {}


def _get_nc(nt):
    if nt not in _NC_CACHE:
        _NC_CACHE[nt] = build_nc(nt)
    return _NC_CACHE[nt]


def _balance_classes(y):
    """LPT-assign classes to cores; returns (assign[C], loads[NCORES])."""
    import heapq
    cnt = np.bincount(y, minlength=C)
    assign = np.full(C, -1, dtype=np.int64)
    heap = [(0, 0, k) for k in range(NCORES)]  # (load, nclasses, core)
    heapq.heapify(heap)
    skipped = []
    for c in np.argsort(-cnt, kind="stable"):
        if cnt[c] == 0:
            break
        load, ncl, k = heapq.heappop(heap)
        if ncl >= NCLS:  # bin full of classes; try others
            skipped.append((load, ncl, k))
            while heap and heap[0][1] >= NCLS:
                skipped.append(heapq.heappop(heap))
            if not heap:
                raise RuntimeError("class balancing failed")
            load, ncl, k = heapq.heappop(heap)
        assign[c] = k
        heapq.heappush(heap, (load + int(cnt[c]), ncl + 1, k))
        for s in skipped:
            heapq.heappush(heap, s)
        skipped = []
    loads = np.zeros(NCORES, dtype=np.int64)
    np.add.at(loads, assign[y], 1)
    return assign, loads


def make_in_maps(xs, y_preds, y_true, nt):
    rb = nt * 128
    xs16 = np.asarray(xs, dtype=np.float16)
    xs8 = np.asarray(xs, dtype=np.float32).astype(ml_dtypes.float8_e4m3)
    yp16 = np.asarray(y_preds, dtype=np.float16)
    y = np.asarray(y_true).astype(np.int64).ravel()
    assert xs8.shape == (B, D) and yp16.shape == (B, C) and y.shape == (B,)

    assign, loads = _balance_classes(y)
    assert loads.max() <= rb, f"bucket overflow: {loads.max()} > {rb}"
    lidx = np.zeros(C, dtype=np.int64)
    for k in range(NCORES):
        cls_k = np.nonzero(assign == k)[0]
        lidx[cls_k] = np.arange(len(cls_k))

    # neg block (fp16): xng[p, k*17+j] = xs[j, k*128+p]
    xng = np.ascontiguousarray(
        xs16[:KNEG].T.reshape(KD, 128, KNEG).transpose(1, 0, 2)
    ).reshape(128, NEGW)

    row_core = assign[y]
    in_maps = []
    for k in range(NCORES):
        rows = np.nonzero(row_core == k)[0]
        nk = len(rows)
        # pad rows are ONES so ln(n2) stays finite; yb=-1 zeroes them out
        xb = np.ones((rb, D), dtype=ml_dtypes.float8_e4m3)
        xb[:nk] = xs8[rows]
        yb = np.full(rb, -1.0, dtype=np.float32)
        yb[:nk] = lidx[y[rows]].astype(np.float32)
        yt = y[k * CE_ROWS : (k + 1) * CE_ROWS].astype(np.float32)
        sm = np.empty((128, nt + 2), dtype=np.float32)
        for t in range(nt):
            sm[:, t] = yb[t * 128 : (t + 1) * 128]
        sm[:, nt] = yt[0:128]
        sm[:, nt + 1] = yt[128:256]
        in_maps.append({
            "xb": xb.reshape(nt, 128, D),
            "yp": yp16[k * CE_ROWS : (k + 1) * CE_ROWS].reshape(CE_T, 128, C),
            "sm": sm,
            "xng": xng,
        })
    return in_maps


def combine(outs):
    """outs: [NCORES][1, OUTW] partial vectors -> final loss scalar."""
    o = np.stack([np.asarray(x, dtype=np.float64).ravel() for x in outs])
    ce_sum = o[:, 0].sum() + o[:, 1].sum()
    g2 = o[:, 2].sum() / GSCALE
    m2 = o[:, 6].sum()
    neg = o[0, 7]
    loss_ce = ce_sum / B
    cnt = (m2 - B) / 2.0
    sum_s = (g2 - B) / 2.0
    pos_sum = cnt - sum_s
    loss_pos = pos_sum / max(cnt, 1.0) if cnt > 0 else 0.0
    loss_neg = neg / (KNEG - 1)
    return np.array(loss_ce + loss_pos + loss_neg, dtype=np.float32)


def kernel(xs, y_preds, y_true, _trace=False):
    y = np.asarray(y_true).astype(np.int64).ravel()
    _, loads = _balance_classes(y)
    nt = max(2, -(-int(loads.max()) // 128))
    nc = _get_nc(nt)
    in_maps = make_in_maps(xs, y_preds, y_true, nt)
    kw = {}
    if _trace:
        import os
        td = "/tmp/trace_out"
        os.makedirs(td, exist_ok=True)
        kw["tmpdir"] = td
    res = bass_utils.run_bass_kernel_spmd(
        nc, in_maps, core_ids=list(range(NCORES)), trace=_trace, **kw,
    )
    loss = combine([r["out"] for r in res.results])
    if _trace:
        return loss, res
    return loss
